# revision 16
# baseline (speedup 1.0000x reference)
"""GCN layer (gather + segment-sum + linear) as a Bass/Tile kernel on 8 trn2 cores.

Strategy (dst-sharded, no collectives):
  - Nodes are split into 8 contiguous dst ranges of 6250; core c owns output
    rows [6250c, 6250(c+1)).
  - Host sorts edges by dst, buckets them per (core, 128-node window, src-half
    group), pads each bucket to a multiple of 128 (idx=0 / scol=254 pads which
    contribute exactly zero through the one-hot), and equalizes bucket sizes
    across cores so all 8 cores run ONE shared SPMD program.
  - On device, per superwindow of 4 windows: gpsimd.dma_gather fetches the
    src rows (bf16, 256B/row) from the replicated node_feats in HBM straight
    into SBUF in [128 edges x ntiles x 128 feats] layout. Calls are chunked
    to 2048 indices and round-robined over the 4 SWDGE queues: descriptor
    generation runs on a queue-selected Q7 core pair, so queues parallelize
    it (it is the dominant cost at ~5ns/descriptor per pair).
  - Segment-sum over a 128-node window = sum of one-hot matmuls on TensorE:
    psum[f, n] += sum_e msg[e, f] * onehot[e, n]. One-hot tiles are
    precomputed on the host in fp8e4m3 and streamed in on the Scalar HWDGE
    queue, which keeps DVE off the critical path and lets PE stream.
  - Linear (out = W @ agg + b) is one more matmul per window (lhsT = W^T),
    bias added during the PSUM->SBUF copy, PE-transposed to [node, feat], and
    DMA'd contiguously into the output slab.
  - The src-half grouping exists because dma_gather indices are int16: group
    lo gathers from node_feats[0:32768), group hi from node_feats[32768:).
"""

import sys

sys.path.insert(0, "/opt/trn_rl_repo")

import numpy as np
import ml_dtypes

BF16 = ml_dtypes.bfloat16
FP8 = ml_dtypes.float8_e4m3

# Problem constants (hardcoded per the harness contract).
N_NODES = 50000
N_FEATS = 128
N_CORES = 8
WIN = 128          # nodes per one-hot window (psum width)
SWS = 4            # windows per superwindow (gather granularity)
SPLIT = 32768      # int16 index limit for dma_gather
PADCOL = 254.0     # one-hot column for padded edge slots (never matches)

CALL_MAX = 2048    # indices per dma_gather call: small enough that calls
                   # round-robin across the 4 SWDGE queues (desc-gen runs on
                   # a queue-selected Q7 core pair, so queues parallelize it)

TRACE = False      # set True from test.py to capture an NTFF profile
LAST = {}          # stash for exec_time_ns etc. when TRACE


def _plan(src, dst, n_nodes, n_cores, win, sws, split):
    """Bucket/pad edges; build per-core data arrays + the shared schedule."""
    npc = n_nodes // n_cores
    nw = -(-npc // win)                      # windows per core
    nsw = -(-nw // sws)                      # superwindows per core

    src = np.asarray(src, dtype=np.int64)
    dst = np.asarray(dst, dtype=np.int64)
    core = dst // npc
    rel = dst - core * npc
    wloc = rel // win
    col = rel % win
    grp = (src >= split).astype(np.int64)

    cnt = np.zeros((n_cores, nw, 2), np.int64)
    np.add.at(cnt, (core, wloc, grp), 1)
    P = (cnt.max(axis=0) + 127) // 128 * 128  # [nw, 2] padded bucket sizes

    order = np.lexsort((grp, wloc, core))
    s_src, s_col = src[order], col[order]
    starts = np.zeros(n_cores * nw * 2 + 1, np.int64)
    np.cumsum(cnt.reshape(-1), out=starts[1:])

    # Shared schedule: stream order per core is sw-major, then group, then window.
    sw_meta = []   # per sw: dict(ntiles, calls=[(g, icol_off, L, t0)], windows=[(w, [(tm, tg)..])])
    NT = int(P.sum()) // 128
    tg_of = np.zeros((nw, 2), np.int64)      # first global tile of bucket (w, g)
    tg = 0
    icol = 0
    for sw in range(nsw):
        ws = list(range(sw * sws, min((sw + 1) * sws, nw)))
        calls = []
        t0 = 0
        sw_tg0 = tg
        for g in (0, 1):
            L = int(sum(P[w, g] for w in ws))
            # chunk to respect the SWDGE descriptor-carveout per call
            a = 0
            while a < L:
                Lc = min(CALL_MAX, L - a)
                calls.append((g, icol, Lc, t0))
                t0 += Lc // 128
                icol += Lc // 16
                a += Lc
            for w in ws:
                tg_of[w, g] = tg
                tg += P[w, g] // 128
        windows = []
        for w in ws:
            tiles = []
            for g in (0, 1):
                for k in range(P[w, g] // 128):
                    gt = int(tg_of[w, g] + k)
                    tiles.append((gt - sw_tg0, gt))
            windows.append((w, tiles))
        sw_meta.append(dict(ntiles=t0, calls=calls, windows=windows))
    assert tg == NT

    # Per-core data arrays.
    per_core = []
    for c in range(n_cores):
        gidx_flat = np.zeros(NT * 128, np.int16)
        scol_flat = np.full(NT * 128, PADCOL, np.float32)
        pos = 0
        for sw in range(nsw):
            ws = list(range(sw * sws, min((sw + 1) * sws, nw)))
            for g in (0, 1):
                for w in ws:
                    b = (c * nw + w) * 2 + g
                    a0, a1 = starts[b], starts[b + 1]
                    n = a1 - a0
                    if n:
                        idxs = s_src[a0:a1] - (split if g else 0)
                        gidx_flat[pos:pos + n] = idxs.astype(np.int16)
                        scol_flat[pos:pos + n] = s_col[a0:a1]
                    pos += int(P[w, g])
        assert pos == NT * 128

        # Wrap indices per gather call: idx i -> [i % 16, i // 16], replicated x8.
        blocks = []
        for sw in range(nsw):
            for (_, _, L, _) in sw_meta[sw]["calls"]:
                a = sum(bl.shape[1] for bl in blocks) * 16
                blk = gidx_flat[a:a + L].reshape(L // 16, 16).T
                blocks.append(blk)
        gidx = np.tile(np.concatenate(blocks, axis=1), (8, 1))
        # one-hot tiles, precomputed: [p(edge-in-tile), tile, node-col]
        sc = scol_flat.reshape(NT, 128).T
        onehot = (sc[:, :, None] == np.arange(win, dtype=np.float32)[None, None, :])
        per_core.append((np.ascontiguousarray(gidx),
                         np.ascontiguousarray(onehot.astype(FP8))))

    meta = dict(n_nodes=n_nodes, n_cores=n_cores, npc=npc, nw=nw, nsw=nsw,
                win=win, split=split, NT=NT, NIC=NT * 8, sw=sw_meta,
                out_rows=nw * win)
    return per_core, meta


def _build(meta):
    """Emit the shared SPMD Tile program."""
    from concourse import bacc, tile, mybir

    dt = mybir.dt
    F = N_FEATS
    nc = bacc.Bacc("TRN2", target_bir_lowering=False, debug=False,
                   num_devices=meta["n_cores"], num_swdge_queues=4)

    nf_d = nc.dram_tensor("nf", [meta["n_nodes"], F], dt.bfloat16, kind="ExternalInput")
    gidx_d = nc.dram_tensor("gidx", [128, meta["NIC"]], dt.int16, kind="ExternalInput")
    oh_d = nc.dram_tensor("onehot", [128, meta["NT"], WIN], dt.float8e4, kind="ExternalInput")
    wt_d = nc.dram_tensor("wt", [F, F], dt.bfloat16, kind="ExternalInput")
    bias_d = nc.dram_tensor("bias", [F, 1], dt.float32, kind="ExternalInput")
    ident_d = nc.dram_tensor("ident", [128, 128], dt.float32, kind="ExternalInput")
    out_d = nc.dram_tensor("out", [meta["out_rows"], F], dt.bfloat16, kind="ExternalOutput")

    max_sw_tiles = max(s["ntiles"] for s in meta["sw"])
    split_rows = meta["split"]
    hi_rows = meta["n_nodes"] - split_rows

    with tile.TileContext(nc) as tc:
        with tc.tile_pool(name="const", bufs=1) as cp, \
             tc.tile_pool(name="msg", bufs=3) as mp, \
             tc.tile_pool(name="oh", bufs=3) as ohp, \
             tc.tile_pool(name="pa", bufs=3, space="PSUM") as pap, \
             tc.tile_pool(name="po", bufs=2, space="PSUM") as pop, \
             tc.tile_pool(name="pt", bufs=2, space="PSUM") as ptp, \
             tc.tile_pool(name="sb", bufs=3) as sbp:

            gidx_t = cp.tile([128, meta["NIC"]], dt.int16, name="gidx_t")
            nc.sync.dma_start(out=gidx_t, in_=gidx_d[:, :])
            wt_t = cp.tile([F, F], dt.bfloat16, name="wt_t")
            nc.sync.dma_start(out=wt_t, in_=wt_d[:, :])
            bias_t = cp.tile([F, 1], dt.float32, name="bias_t")
            nc.sync.dma_start(out=bias_t, in_=bias_d[:, :])
            ident_t = cp.tile([128, 128], dt.float32, name="ident_t")
            nc.sync.dma_start(out=ident_t, in_=ident_d[:, :])

            warm_idx = cp.tile([128, 8], dt.int16, name="warm_idx")
            nc.gpsimd.memset(warm_idx, 0)
            warm_out = cp.tile([128, 4, 128], dt.bfloat16, name="warm_out")
            for q in range(4):
                nc.gpsimd.dma_gather(
                    warm_out[:, q:q + 1, :], nf_d[0:128, :], warm_idx,
                    128, 128, F, single_packet=False, queue_num=q)

            call_no = 0
            sw_t0 = 0
            for sw, sm in enumerate(meta["sw"]):
                msg = mp.tile([128, max_sw_tiles, 128], dt.bfloat16, name="msg")
                oh = ohp.tile([128, max_sw_tiles, WIN], dt.float8e4, name="oh")
                nc.scalar.dma_start(
                    out=oh[:, 0:sm["ntiles"], :],
                    in_=oh_d[:, sw_t0:sw_t0 + sm["ntiles"], :])
                for (g, icol, L, t0) in sm["calls"]:
                    src_ap = nf_d[split_rows:split_rows + hi_rows, :] if g \
                        else nf_d[0:split_rows, :]
                    nc.gpsimd.dma_gather(
                        msg[:, t0:t0 + L // 128, :],
                        src_ap,
                        gidx_t[:, icol:icol + L // 16],
                        L, L, F,
                        single_packet=False,
                        queue_num=call_no % 4,
                    )
                    call_no += 1
                for (w, tiles) in sm["windows"]:
                    pa = pap.tile([128, WIN], dt.float32, name="pa")
                    last = len(tiles) - 1
                    for k, (tm, gt) in enumerate(tiles):
                        nc.tensor.matmul(pa, msg[:, tm:tm + 1, :],
                                         oh[:, tm:tm + 1, :],
                                         start=(k == 0), stop=(k == last))
                    aggsb = sbp.tile([F, WIN], dt.bfloat16, name="aggsb", tag="aggsb")
                    nc.vector.tensor_copy(aggsb, pa)
                    po = pop.tile([F, WIN], dt.float32, name="po")
                    nc.tensor.matmul(po, wt_t, aggsb, start=True, stop=True)
                    outsb = sbp.tile([F, WIN], dt.float32, name="outsb", tag="outsb")
                    nc.vector.tensor_scalar_add(outsb, po, bias_t[:, 0:1])
                    pt = ptp.tile([WIN, F], dt.float32, name="pt")
                    nc.tensor.transpose(pt, outsb, ident_t)
                    fin = sbp.tile([WIN, F], dt.bfloat16, name="fin", tag="fin")
                    nc.vector.tensor_copy(fin, pt)
                    nc.sync.dma_start(out=out_d[w * WIN:(w + 1) * WIN, :], in_=fin)
                sw_t0 += sm["ntiles"]

    nc.compile()
    return nc


def _in_maps(node_feats, W, b, per_core, meta):
    nf = np.ascontiguousarray(node_feats.astype(BF16))
    wt = np.ascontiguousarray(W.T.astype(BF16))
    bias = np.ascontiguousarray(b.reshape(-1, 1).astype(np.float32))
    ident = np.eye(128, dtype=np.float32)
    maps = []
    for (gidx, onehot) in per_core:
        maps.append(dict(nf=nf, gidx=gidx, onehot=onehot, wt=wt, bias=bias,
                         ident=ident))
    return maps


def _run_traced(nc, maps, n_cores):
    """Execute via PJRT with an NTFF profile captured locally (dev only)."""
    import glob
    import tempfile

    import gauge.profiler
    from concourse import bass2jax
    from concourse._compat import FishPath
    from trn_agent_boot.trn_boot import _ntff_profile_via_ctypes

    hookf = _ntff_profile_via_ctypes("/opt/axon/libaxon_pjrt.so")
    assert hookf is not None, "libaxon_pjrt.so too old for NTFF profiling"
    neff_dir = tempfile.mkdtemp(prefix="gcn_ntff_")
    with hookf(neff_dir, [0]):
        results = bass2jax.run_bass_via_pjrt(nc, maps, n_cores=n_cores)
    ntffs = glob.glob(neff_dir + "/*_body*.ntff")
    LAST["neff_dir"] = neff_dir
    if not ntffs:
        print(f"WARNING: no NTFFs in {neff_dir}: {sorted(glob.os.listdir(neff_dir))}")
        return results
    profile = gauge.profiler.Profile(
        profile_path=FishPath(neff_dir),
        kernel_dev_mode=True,
        profile_on_exit=False,
        bass_kernel=nc.m,
        offline_processing=True,
        fname="*_body*",
    )
    pr = profile.to_perfetto(model_index=(0,))
    if pr:
        LAST["exec_time_ns"] = pr[0].exec_time_ns
        LAST["trace_path"] = pr[0].trace_path
        LAST["insts"] = pr[0].insts
    return results


def kernel(node_feats, edge_feats, src, dst, W, b):
    from concourse import bass2jax

    node_feats = np.asarray(node_feats, dtype=np.float32)
    W = np.asarray(W, dtype=np.float32)
    b = np.asarray(b, dtype=np.float32)

    per_core, meta = _plan(src, dst, N_NODES, N_CORES, WIN, SWS, SPLIT)
    nc = _build(meta)
    maps = _in_maps(node_feats, W, b, per_core, meta)

    if TRACE:
        results = _run_traced(nc, maps, N_CORES)
    else:
        results = bass2jax.run_bass_via_pjrt(nc, maps, n_cores=N_CORES)

    npc = meta["npc"]
    out = np.concatenate(
        [np.asarray(results[c]["out"][:npc]).astype(np.float32)
         for c in range(N_CORES)], axis=0)
    return np.ascontiguousarray(out)
